# revision 16
# baseline (speedup 1.0000x reference)
"""Trainium2 kernel for nn_CropRandomizer_9062380994640.

Problem: images [64,3,224,224] f32 + crop_inds [64,8,2] int32 ->
8 crops of 192x192 per image -> out [512,3,192,192] f32.

Sharding: pure data parallel — 8 images (64 crops) per NeuronCore, 8 cores.

Per-core pipeline (all descriptors are large; no per-row HBM descriptors):
  0. A prologue casts each image f32 -> bf16 into a DRAM scratch via
     gpsimd (SWDGE) cast-DMAs, one per image, halving all re-read traffic
     (the 8 crops per image overlap ~73%, so the gather re-reads ~7x).
     bf16 rounding gives ~0.2% rel err, far under the 2% gate.
  1. crop_inds are DMA-broadcast into a [96,32] SBUF tile (one quarter of
     the partitions per 4-crop "slot"), and a static per-partition offset
     table poff[96,16] is loaded.
  2. The vector engine computes 96 gather offsets per group of 4 crops:
     idx[p,g] = r*W + q + poff[p,g], where poff bakes in the image index,
     channel and row-block of partition p (all static).
  3. For each of 16 groups, one gpsimd indirect DMA gathers 96 contiguous
     5376-element bf16 runs (24 rows of 224, already shifted by the crop's
     column offset q) from the scratch into a [96,5376] slab. The column
     shift is folded into the gather offset, so each run is contiguous.
  4. The vector engine repacks [96, 24x224 -> 24x192] with a static access
     pattern (drops the 32 pad columns per row), staying in bf16 (2x DVE
     rate, half the SBUF bytes).
  5. One SWDGE cast-DMA per TWO groups stores packed bf16 -> f32 output
     (8 crops, 1.77 MB of SBUF reads -> 3.5 MB of HBM writes). Keeping
     the SBUF side bf16 halves the store's SBUF-read traffic, which is
     the binding resource (~435 GB/s per direction per core).
Stages are double/triple-buffered with per-buffer-slot semaphores (DMA
completions are unordered across a queue, so each slot gets its own sem).
"""
import numpy as np
from concourse import bass, bacc, mybir
from concourse.bass_utils import run_bass_kernel_spmd

M = 8  # cores
B, C, H, W = 64, 3, 224, 224
N = 8
CH = CW = 192
B_LOC = B // M           # images per core
U = B_LOC * N            # crops per core
CHW = C * H * W
HW = H * W
G = 4                    # crops per gather group
NGRP = U // G            # 16 groups
SLAB_P = 96              # partitions per group (24 per crop)
SLAB_F = G * C * CH * W // SLAB_P    # 5376 = 24 rows of 224
PACK_F = G * C * CH * CW // SLAB_P   # 4608 = 24 rows of 192
NBUF = 4                 # groups in flight

_nc = None
LAST_RESULT = None


def _poff_table() -> np.ndarray:
    """poff[p, g] = b*CHW + c*HW + (row-block of p)*24*W for crop u=4g+p//24.
    Static part of the gather offset (crop_inds contribute r*W + q)."""
    poff = np.zeros((SLAB_P, NGRP), np.int32)
    for g in range(NGRP):
        for p in range(SLAB_P):
            u = g * G + p // 24
            b = u // N
            c = (p % 24) // 8
            k = p % 8
            poff[p, g] = b * CHW + c * HW + k * 24 * W
    return poff


STORE_BF16 = True  # False: f32 packed + HWDGE stores on sync/scalar


def _build(repeat=1, store_bf16=None):
    if store_bf16 is None:
        store_bf16 = STORE_BF16
    nc = bacc.Bacc()
    images = nc.dram_tensor(
        "images", [B_LOC, C, H, W], mybir.dt.float32, kind="ExternalInput"
    )
    crop_inds = nc.dram_tensor(
        "crop_inds", [B_LOC, N, 2], mybir.dt.int32, kind="ExternalInput"
    )
    poff = nc.dram_tensor("poff", [SLAB_P, NGRP], mybir.dt.int32, kind="ExternalInput")
    out = nc.dram_tensor("out", [U, C, CH, CW], mybir.dt.float32, kind="ExternalOutput")
    img16 = nc.dram_tensor("img16", [B_LOC, C, H, W], mybir.dt.bfloat16, kind="Internal")
    images2d = images.rearrange("b c h w -> (b c) (h w)")
    img16_2d = img16.rearrange("b c h w -> (b c) (h w)")
    out_flat = out.rearrange("u c h w -> (u c h w)")
    ci_flat = crop_inds.rearrange("b n t -> (b n t)")

    with (
        nc.sbuf_tensor("cib", [SLAB_P, 2 * NGRP], mybir.dt.int32) as cib,
        nc.sbuf_tensor("poffs", [SLAB_P, NGRP], mybir.dt.int32) as poffs,
        nc.sbuf_tensor("idxs", [SLAB_P, NGRP], mybir.dt.int32) as idxs,
        nc.sbuf_tensor("slab", [SLAB_P, NBUF * SLAB_F], mybir.dt.bfloat16) as slab,
        nc.sbuf_tensor(
            "packed",
            [SLAB_P, NBUF * PACK_F],
            mybir.dt.bfloat16 if store_bf16 else mybir.dt.float32,
        ) as packed,
        nc.semaphore("in_sem") as in_sem,
        nc.semaphore("cast_sem") as cast_sem,
        nc.semaphore("idx_sem") as idx_sem,
        nc.semaphore("vv_sem") as vv_sem,
        nc.semaphore("ld0") as ld0,
        nc.semaphore("ld1") as ld1,
        nc.semaphore("ld2") as ld2,
        nc.semaphore("ld3") as ld3,
        nc.semaphore("rp_sem") as rp_sem,
        nc.semaphore("st0") as st0,
        nc.semaphore("st1") as st1,
        nc.semaphore("st2") as st2,
        nc.semaphore("st3") as st3,
        nc.Block() as block,
    ):
        lds = [ld0, ld1, ld2, ld3]
        # bf16 mode: paired stores, one sem per buffer PAIR ({0,1}->st0, {2,3}->st1)
        # f32 mode: per-group stores, one sem per buffer slot
        stp = [st0, st1]
        sts = [st0, st1, st2, st3]
        NCAST = 4  # cast-DMAs, 2 images each

        def issue_store(gp, k):
            # store k covers groups 2k, 2k+1 (bufs 2k%4, 2k%4+1), casting
            # packed bf16 -> f32 output via SWDGE
            g0 = (2 * k) % NGRP
            b0 = (2 * k) % NBUF
            gp.wait_ge(rp_sem, 2 * k + 2)
            src = bass.AP(
                packed,
                b0 * PACK_F,
                [[NBUF * PACK_F, SLAB_P], [PACK_F, 2], [1, PACK_F]],
            )
            dst = bass.AP(
                out_flat.tensor,
                g0 * G * C * CH * CW,
                [[PACK_F, SLAB_P], [SLAB_P * PACK_F, 2], [1, PACK_F]],
            )
            gp.dma_start(dst, src).then_inc(stp[k % 2], 16)

        def issue_store_f32(eng, n):
            # one HWDGE f32 store per group (baseline-style)
            g = n % NGRP
            buf = n % NBUF
            eng.wait_ge(rp_sem, n + 1)
            src = packed[:, buf * PACK_F : (buf + 1) * PACK_F]
            dst = bass.AP(
                out_flat.tensor,
                g * G * C * CH * CW,
                [[PACK_F, SLAB_P], [1, PACK_F]],
            )
            eng.dma_start(dst, src).then_inc(sts[buf], 16)

        @block.sync
        def _(sync):
            # Broadcast crop_inds into 4 partition quarters: partition p gets
            # the (r, q) pairs of crop u = 4g + p//24, g = 0..15.
            for quarter in range(G):
                src = bass.AP(
                    crop_inds, 2 * quarter, [[0, 24], [2 * G, NGRP], [1, 2]]
                )
                sync.dma_start(
                    cib[24 * quarter : 24 * (quarter + 1), :], src
                ).then_inc(in_sem, 16)
            sync.dma_start(poffs[:, :], poff[:, :]).then_inc(in_sem, 16)
            if not store_bf16:
                for n in range(NGRP * repeat):
                    if n % 2 == 1:
                        issue_store_f32(sync, n)

        @block.vector
        def _(vec):
            vec.wait_ge(in_sem, 16 * 5)
            r_view = bass.AP(cib, 0, [[2 * NGRP, SLAB_P], [2, NGRP]])
            q_view = bass.AP(cib, 1, [[2 * NGRP, SLAB_P], [2, NGRP]])
            vec.tensor_scalar_mul(idxs[:, :], r_view, W).then_inc(vv_sem, 1)
            vec.wait_ge(vv_sem, 1)
            vec.tensor_tensor(
                out=idxs[:, :], in0=idxs[:, :], in1=q_view, op=mybir.AluOpType.add
            ).then_inc(vv_sem, 1)
            vec.wait_ge(vv_sem, 2)
            vec.tensor_tensor(
                out=idxs[:, :], in0=idxs[:, :], in1=poffs[:, :],
                op=mybir.AluOpType.add,
            ).then_inc(idx_sem, 1)
            # repack loop
            for n in range(NGRP * repeat):
                buf = n % NBUF
                vec.wait_ge(lds[buf], 16 * (n // NBUF + 1))
                src = bass.AP(
                    slab,
                    buf * SLAB_F,
                    [[NBUF * SLAB_F, SLAB_P], [W, SLAB_F // W], [1, CW]],
                )
                dst = bass.AP(
                    packed,
                    buf * PACK_F,
                    [[NBUF * PACK_F, SLAB_P], [CW, PACK_F // CW], [1, CW]],
                )
                vec.tensor_copy(dst, src).then_inc(rp_sem, 1)

        @block.gpsimd
        def _(gp):
            total = NGRP * repeat
            # f32 -> bf16 cast pass: NCAST SWDGE cast-DMAs, 2 images each
            per = B_LOC // NCAST
            for i in range(NCAST):
                gp.dma_start(
                    img16_2d[3 * per * i : 3 * per * (i + 1), :],
                    images2d[3 * per * i : 3 * per * (i + 1), :],
                ).then_inc(cast_sem, 16)
            gp.wait_ge(idx_sem, 1)
            for n in range(total):
                g = n % NGRP
                buf = n % NBUF
                if n >= NBUF:
                    if store_bf16:
                        gp.wait_ge(stp[buf // 2], 16 * (n // NBUF))
                    else:
                        gp.wait_ge(sts[buf], 16 * (n // NBUF))
                if n < NGRP:
                    # group g covers 4 crops of image g//2; cast i covers
                    # images 2i, 2i+1 -> groups 4i..4i+3
                    gp.wait_ge(cast_sem, 16 * (g // 4 + 1))
                gp.indirect_dma_start(
                    out=slab[:, buf * SLAB_F : (buf + 1) * SLAB_F],
                    out_offset=None,
                    in_=img16_2d[:],
                    in_offset=bass.IndirectOffsetOnAxis(
                        ap=idxs[:, g : g + 1], axis=1
                    ),
                ).then_inc(lds[buf], 16)
                # store k lags the gather front by ~2 groups
                if store_bf16 and n % 2 == 1 and n >= 3:
                    issue_store(gp, (n - 3) // 2)
            if store_bf16:
                issue_store(gp, total // 2 - 1)
            for b_ in range(NBUF):
                gp.wait_ge(lds[b_], 16 * ((total + NBUF - 1 - b_) // NBUF))
            if store_bf16:
                for p_ in range(2):
                    gp.wait_ge(stp[p_], 16 * (total // 4))

        if not store_bf16:

            @block.scalar
            def _(scalar):
                total = NGRP * repeat
                for n in range(total):
                    if n % 2 == 0:
                        issue_store_f32(scalar, n)
                for b_ in range(NBUF):
                    scalar.wait_ge(
                        sts[b_], 16 * ((total + NBUF - 1 - b_) // NBUF)
                    )

    nc.finalize()
    return nc


def kernel(images: np.ndarray, crop_inds: np.ndarray) -> np.ndarray:
    global _nc, LAST_RESULT
    if _nc is None:
        _nc = _build()
    images = np.ascontiguousarray(images, dtype=np.float32)
    crop_inds = np.ascontiguousarray(crop_inds, dtype=np.int32)
    poff = _poff_table()
    in_maps = [
        {
            "images": images[m * B_LOC : (m + 1) * B_LOC],
            "crop_inds": crop_inds[m * B_LOC : (m + 1) * B_LOC],
            "poff": poff,
        }
        for m in range(M)
    ]
    LAST_RESULT = run_bass_kernel_spmd(_nc, in_maps, core_ids=list(range(M)))
    return np.concatenate(
        [LAST_RESULT.results[m]["out"] for m in range(M)], axis=0
    )



# revision 17
# speedup vs baseline: 1.1375x; 1.1375x over previous
"""Trainium2 kernel for nn_CropRandomizer_9062380994640.

Problem: images [64,3,224,224] f32 + crop_inds [64,8,2] int32 ->
8 crops of 192x192 per image -> out [512,3,192,192] f32.

Sharding: pure data parallel — 8 images (64 crops) per NeuronCore, 8 cores.

Per-core pipeline (all descriptors are large; no per-row HBM descriptors):
  0. A prologue casts each image f32 -> bf16 into a DRAM scratch via
     gpsimd (SWDGE) cast-DMAs, one per image, halving all re-read traffic
     (the 8 crops per image overlap ~73%, so the gather re-reads ~7x).
     bf16 rounding gives ~0.2% rel err, far under the 2% gate.
  1. crop_inds are DMA-broadcast into a [96,32] SBUF tile (one quarter of
     the partitions per 4-crop "slot"), and a static per-partition offset
     table poff[96,16] is loaded.
  2. The vector engine computes 96 gather offsets per group of 4 crops:
     idx[p,g] = r*W + q + poff[p,g], where poff bakes in the image index,
     channel and row-block of partition p (all static).
  3. For each of 16 groups, one gpsimd indirect DMA gathers 96 contiguous
     5376-element bf16 runs (24 rows of 224, already shifted by the crop's
     column offset q) from the scratch into a [96,5376] slab. The column
     shift is folded into the gather offset, so each run is contiguous.
  4. The vector engine repacks [96, 24x224 -> 24x192] with a static access
     pattern (drops the 32 pad columns per row), staying in bf16 (2x DVE
     rate, half the SBUF bytes).
  5. One SWDGE cast-DMA per TWO groups stores packed bf16 -> f32 output
     (8 crops, 1.77 MB of SBUF reads -> 3.5 MB of HBM writes). Keeping
     the SBUF side bf16 halves the store's SBUF-read traffic, which is
     the binding resource (~435 GB/s per direction per core).
Stages are double/triple-buffered with per-buffer-slot semaphores (DMA
completions are unordered across a queue, so each slot gets its own sem).
"""
import numpy as np
from concourse import bass, bacc, mybir
from concourse.bass_utils import run_bass_kernel_spmd

M = 8  # cores
B, C, H, W = 64, 3, 224, 224
N = 8
CH = CW = 192
B_LOC = B // M           # images per core
U = B_LOC * N            # crops per core
CHW = C * H * W
HW = H * W
G = 4                    # crops per gather group
NGRP = U // G            # 16 groups
SLAB_P = 96              # partitions per group (24 per crop)
SLAB_F = G * C * CH * W // SLAB_P    # 5376 = 24 rows of 224
PACK_F = G * C * CH * CW // SLAB_P   # 4608 = 24 rows of 192
NBUF = 4                 # groups in flight

_nc = None
LAST_RESULT = None


def _poff_table() -> np.ndarray:
    """poff[p, g] = b*CHW + c*HW + (row-block of p)*24*W for crop u=4g+p//24.
    Static part of the gather offset (crop_inds contribute r*W + q)."""
    poff = np.zeros((SLAB_P, NGRP), np.int32)
    for g in range(NGRP):
        for p in range(SLAB_P):
            u = g * G + p // 24
            b = u // N
            c = (p % 24) // 8
            k = p % 8
            poff[p, g] = b * CHW + c * HW + k * 24 * W
    return poff


STORE_BF16 = False  # False: f32 packed + HWDGE stores on sync/scalar


def _build(repeat=1, store_bf16=None):
    if store_bf16 is None:
        store_bf16 = STORE_BF16
    nc = bacc.Bacc()
    images = nc.dram_tensor(
        "images", [B_LOC, C, H, W], mybir.dt.float32, kind="ExternalInput"
    )
    crop_inds = nc.dram_tensor(
        "crop_inds", [B_LOC, N, 2], mybir.dt.int32, kind="ExternalInput"
    )
    poff = nc.dram_tensor("poff", [SLAB_P, NGRP], mybir.dt.int32, kind="ExternalInput")
    out = nc.dram_tensor("out", [U, C, CH, CW], mybir.dt.float32, kind="ExternalOutput")
    img16 = nc.dram_tensor("img16", [B_LOC, C, H, W], mybir.dt.bfloat16, kind="Internal")
    images2d = images.rearrange("b c h w -> (b c) (h w)")
    img16_2d = img16.rearrange("b c h w -> (b c) (h w)")
    out_flat = out.rearrange("u c h w -> (u c h w)")
    ci_flat = crop_inds.rearrange("b n t -> (b n t)")

    with (
        nc.sbuf_tensor("cib", [SLAB_P, 2 * NGRP], mybir.dt.int32) as cib,
        nc.sbuf_tensor("poffs", [SLAB_P, NGRP], mybir.dt.int32) as poffs,
        nc.sbuf_tensor("idxs", [SLAB_P, NGRP], mybir.dt.int32) as idxs,
        nc.sbuf_tensor("slab", [SLAB_P, NBUF * SLAB_F], mybir.dt.bfloat16) as slab,
        nc.sbuf_tensor(
            "packed",
            [SLAB_P, NBUF * PACK_F],
            mybir.dt.bfloat16 if store_bf16 else mybir.dt.float32,
        ) as packed,
        nc.semaphore("in_sem") as in_sem,
        nc.semaphore("cast_sem") as cast_sem,
        nc.semaphore("idx_sem") as idx_sem,
        nc.semaphore("vv_sem") as vv_sem,
        nc.semaphore("ld0") as ld0,
        nc.semaphore("ld1") as ld1,
        nc.semaphore("ld2") as ld2,
        nc.semaphore("ld3") as ld3,
        nc.semaphore("rp_sem") as rp_sem,
        nc.semaphore("st0") as st0,
        nc.semaphore("st1") as st1,
        nc.semaphore("st2") as st2,
        nc.semaphore("st3") as st3,
        nc.Block() as block,
    ):
        lds = [ld0, ld1, ld2, ld3]
        # bf16 mode: paired stores, one sem per buffer PAIR ({0,1}->st0, {2,3}->st1)
        # f32 mode: per-group stores, one sem per buffer slot
        stp = [st0, st1]
        sts = [st0, st1, st2, st3]
        NCAST = 4  # cast-DMAs, 2 images each

        def issue_store(gp, k):
            # store k covers groups 2k, 2k+1 (bufs 2k%4, 2k%4+1), casting
            # packed bf16 -> f32 output via SWDGE
            g0 = (2 * k) % NGRP
            b0 = (2 * k) % NBUF
            gp.wait_ge(rp_sem, 2 * k + 2)
            src = bass.AP(
                packed,
                b0 * PACK_F,
                [[NBUF * PACK_F, SLAB_P], [PACK_F, 2], [1, PACK_F]],
            )
            dst = bass.AP(
                out_flat.tensor,
                g0 * G * C * CH * CW,
                [[PACK_F, SLAB_P], [SLAB_P * PACK_F, 2], [1, PACK_F]],
            )
            gp.dma_start(dst, src).then_inc(stp[k % 2], 16)

        def issue_store_f32(eng, n):
            # one HWDGE f32 store per group (baseline-style)
            g = n % NGRP
            buf = n % NBUF
            eng.wait_ge(rp_sem, n + 1)
            src = packed[:, buf * PACK_F : (buf + 1) * PACK_F]
            dst = bass.AP(
                out_flat.tensor,
                g * G * C * CH * CW,
                [[PACK_F, SLAB_P], [1, PACK_F]],
            )
            eng.dma_start(dst, src).then_inc(sts[buf], 16)

        @block.sync
        def _(sync):
            # Broadcast crop_inds into 4 partition quarters: partition p gets
            # the (r, q) pairs of crop u = 4g + p//24, g = 0..15.
            for quarter in range(G):
                src = bass.AP(
                    crop_inds, 2 * quarter, [[0, 24], [2 * G, NGRP], [1, 2]]
                )
                sync.dma_start(
                    cib[24 * quarter : 24 * (quarter + 1), :], src
                ).then_inc(in_sem, 16)
            sync.dma_start(poffs[:, :], poff[:, :]).then_inc(in_sem, 16)
            if not store_bf16:
                for n in range(NGRP * repeat):
                    if n % 2 == 1:
                        issue_store_f32(sync, n)

        @block.vector
        def _(vec):
            vec.wait_ge(in_sem, 16 * 5)
            r_view = bass.AP(cib, 0, [[2 * NGRP, SLAB_P], [2, NGRP]])
            q_view = bass.AP(cib, 1, [[2 * NGRP, SLAB_P], [2, NGRP]])
            vec.tensor_scalar_mul(idxs[:, :], r_view, W).then_inc(vv_sem, 1)
            vec.wait_ge(vv_sem, 1)
            vec.tensor_tensor(
                out=idxs[:, :], in0=idxs[:, :], in1=q_view, op=mybir.AluOpType.add
            ).then_inc(vv_sem, 1)
            vec.wait_ge(vv_sem, 2)
            vec.tensor_tensor(
                out=idxs[:, :], in0=idxs[:, :], in1=poffs[:, :],
                op=mybir.AluOpType.add,
            ).then_inc(idx_sem, 1)
            # repack loop
            for n in range(NGRP * repeat):
                buf = n % NBUF
                vec.wait_ge(lds[buf], 16 * (n // NBUF + 1))
                src = bass.AP(
                    slab,
                    buf * SLAB_F,
                    [[NBUF * SLAB_F, SLAB_P], [W, SLAB_F // W], [1, CW]],
                )
                dst = bass.AP(
                    packed,
                    buf * PACK_F,
                    [[NBUF * PACK_F, SLAB_P], [CW, PACK_F // CW], [1, CW]],
                )
                vec.tensor_copy(dst, src).then_inc(rp_sem, 1)

        @block.gpsimd
        def _(gp):
            total = NGRP * repeat
            # f32 -> bf16 cast pass: NCAST SWDGE cast-DMAs, 2 images each
            per = B_LOC // NCAST
            for i in range(NCAST):
                gp.dma_start(
                    img16_2d[3 * per * i : 3 * per * (i + 1), :],
                    images2d[3 * per * i : 3 * per * (i + 1), :],
                ).then_inc(cast_sem, 16)
            gp.wait_ge(idx_sem, 1)
            for n in range(total):
                g = n % NGRP
                buf = n % NBUF
                if n >= NBUF:
                    if store_bf16:
                        gp.wait_ge(stp[buf // 2], 16 * (n // NBUF))
                    else:
                        gp.wait_ge(sts[buf], 16 * (n // NBUF))
                if n < NGRP:
                    # group g covers 4 crops of image g//2; cast i covers
                    # images 2i, 2i+1 -> groups 4i..4i+3
                    gp.wait_ge(cast_sem, 16 * (g // 4 + 1))
                gp.indirect_dma_start(
                    out=slab[:, buf * SLAB_F : (buf + 1) * SLAB_F],
                    out_offset=None,
                    in_=img16_2d[:],
                    in_offset=bass.IndirectOffsetOnAxis(
                        ap=idxs[:, g : g + 1], axis=1
                    ),
                ).then_inc(lds[buf], 16)
                # store k lags the gather front by ~2 groups
                if store_bf16 and n % 2 == 1 and n >= 3:
                    issue_store(gp, (n - 3) // 2)
            if store_bf16:
                issue_store(gp, total // 2 - 1)
            for b_ in range(NBUF):
                gp.wait_ge(lds[b_], 16 * ((total + NBUF - 1 - b_) // NBUF))
            if store_bf16:
                for p_ in range(2):
                    gp.wait_ge(stp[p_], 16 * (total // 4))

        if not store_bf16:

            @block.scalar
            def _(scalar):
                total = NGRP * repeat
                for n in range(total):
                    if n % 2 == 0:
                        issue_store_f32(scalar, n)
                for b_ in range(NBUF):
                    scalar.wait_ge(
                        sts[b_], 16 * ((total + NBUF - 1 - b_) // NBUF)
                    )

    nc.finalize()
    return nc


def kernel(images: np.ndarray, crop_inds: np.ndarray) -> np.ndarray:
    global _nc, LAST_RESULT
    if _nc is None:
        _nc = _build()
    images = np.ascontiguousarray(images, dtype=np.float32)
    crop_inds = np.ascontiguousarray(crop_inds, dtype=np.int32)
    poff = _poff_table()
    in_maps = [
        {
            "images": images[m * B_LOC : (m + 1) * B_LOC],
            "crop_inds": crop_inds[m * B_LOC : (m + 1) * B_LOC],
            "poff": poff,
        }
        for m in range(M)
    ]
    LAST_RESULT = run_bass_kernel_spmd(_nc, in_maps, core_ids=list(range(M)))
    return np.concatenate(
        [LAST_RESULT.results[m]["out"] for m in range(M)], axis=0
    )

